# revision 9
# baseline (speedup 1.0000x reference)
"""Trainium2 Bass kernel: batched 4-point DLT homography (closed-form solve).

Contract: kernel(pts_1_tile, pred_h4p_tile) -> [B, 3, 3] float32, with
B = 524288 split across 8 NeuronCores (batch-parallel, no communication).

Math (per batch element, points p=0..3 with src (x_p,y_p), dst (X_p,Y_p)):
the DLT system rows are
    x h0 + y h1 + h2 = X (1 + x h6 + y h7)
    x h3 + y h4 + h5 = Y (1 + x h6 + y h7)
Eliminating (h0,h1,h2) from the four X-equations via the left null vector n
of M = [(x_p, y_p, 1)] gives one linear equation in (h6,h7); same for the
Y-equations. Solve the 2x2, back out the rest in closed form.

Layout: per-core 65536 elements as [128 partitions, 512 free], two
asymmetric chunks (128 + 384 columns) so chunk-0 compute starts once a
quarter of the input lands.  I/O is fp16 (host casts).  Planes are [128,fc]
scalars in a no-reuse slab (avoids WAR-hazard semaphores).  Engine split,
calibrated from HW traces: VectorE owns all math at fp16-2x (0.52 ns/elem);
ScalarE does the v-deinterleave + first u-half (fp16 copies run at only
~2 ns/elem on ACT); GPSIMD does the ones-memset + second u-half.  Final h
values are written by VectorE directly into the interleaved output staging
(strided dst, 1x) - no separate output copies.  Reciprocals of n3 and det
are fused into one two-plane fp32 op and consumed as fp32 broadcasts.
"""
import sys

for _p in ("/opt/trn_rl_repo", "/root/.axon_site/_ro/trn_rl_repo"):
    if _p not in sys.path:
        sys.path.append(_p)

import numpy as np

import concourse.bass as bass
import concourse.mybir as mybir
from concourse import bacc
from concourse.tile import TileContext
from concourse.bass_utils import run_bass_kernel_spmd

N_CORES = 8
B_TOTAL = 524288
PER_CORE = B_TOTAL // N_CORES  # 65536
PARTS = 128
F = PER_CORE // PARTS  # 512
CHUNKS = [128, 384]
FP32 = mybir.dt.float32
FP16 = mybir.dt.float16

ADD = mybir.AluOpType.add
SUB = mybir.AluOpType.subtract
MUL = mybir.AluOpType.mult


class _Slab:
    """Bump allocator, F-plane units, no reuse (avoids WAR semaphores)."""

    def __init__(self, nplanes):
        self.off = 0
        self.nplanes = nplanes

    def alloc(self, n):
        off = self.off
        self.off += n
        if self.off > self.nplanes:
            raise RuntimeError(f"slab OOM at {self.off}/{self.nplanes}")
        return off


OPLOG = {}


def _build():
    OPLOG.clear()
    nc = bacc.Bacc(None, target_bir_lowering=False, debug=True)
    pts = nc.dram_tensor("pts", [PER_CORE, 8], FP16, kind="ExternalInput")
    prd = nc.dram_tensor("prd", [PER_CORE, 8], FP16, kind="ExternalInput")
    out = nc.dram_tensor("out", [PER_CORE, 9], FP16, kind="ExternalOutput")

    N32 = 6   # fp32 slab: [n3_32, det_32, rD_32, rdet_32, scratch x2]
    NP = 120  # fp16 compute-plane slab (no reuse)

    with TileContext(nc) as tc:
        with tc.tile_pool(name="st", bufs=1) as pool:
            tiles = {}
            for c, fc in enumerate(CHUNKS):
                tiles[c] = {
                    "vt": pool.tile([PARTS, 8 * fc], FP16, tag=f"vt{c}", name=f"vt{c}"),
                    "pt": pool.tile([PARTS, 8 * fc], FP16, tag=f"pt{c}", name=f"pt{c}"),
                    "ut": pool.tile([PARTS, 8 * fc], FP16, tag=f"ut{c}", name=f"ut{c}"),
                    "ot": pool.tile([PARTS, 9 * fc], FP16, tag=f"ot{c}", name=f"ot{c}"),
                    "s32": pool.tile([PARTS, N32 * fc], FP32, tag=f"s32_{c}", name=f"s32_{c}"),
                    "sp": pool.tile([PARTS, NP * fc], FP16, tag=f"sp{c}", name=f"sp{c}"),
                }

            # All input DMAs up front, chunk 0 first (smallest => earliest start)
            lo = 0
            for c, fc in enumerate(CHUNKS):
                hi = lo + PARTS * fc
                nc.sync.dma_start(
                    out=tiles[c]["vt"][:, :],
                    in_=pts[lo:hi, :].rearrange("(p f) c -> p (f c)", p=PARTS),
                )
                nc.sync.dma_start(
                    out=tiles[c]["pt"][:, :],
                    in_=prd[lo:hi, :].rearrange("(p f) c -> p (f c)", p=PARTS),
                )
                lo = hi

            lo = 0
            for c, fc in enumerate(CHUNKS):
                hi = lo + PARTS * fc
                vt, pt, ut, ot = (tiles[c][k] for k in ("vt", "pt", "ut", "ot"))
                slab32, slabp = tiles[c]["s32"], tiles[c]["sp"]
                sa = _Slab(NP)

                def R32(off, n):
                    return slab32[:, off * fc : (off + n) * fc]

                def R(off, n):
                    return slabp[:, off * fc : (off + n) * fc]

                def V(off, n):
                    return R(off, n).rearrange("p (c f) -> p c f", f=fc)

                def PL(off):
                    return R(off, 1)

                def BC(off, k):
                    return PL(off).unsqueeze(1).broadcast_to((PARTS, k, fc))

                def BC32(off, k):
                    return (
                        R32(off, 1).unsqueeze(1).broadcast_to((PARTS, k, fc))
                    )

                def vtt(o, a, b, op, desc=""):
                    ins = nc.vector.tensor_tensor(out=o, in0=a, in1=b, op=op)
                    OPLOG[ins.ins.name] = desc or "tt"

                def gtt(o, a, b, op, desc=""):
                    ins = nc.gpsimd.tensor_tensor(out=o, in0=a, in1=b, op=op)
                    OPLOG[ins.ins.name] = desc or "gtt"

                def stt(o, in0, scalar, in1, op0, op1, desc="stt"):
                    ins = nc.vector.scalar_tensor_tensor(
                        out=o, in0=in0, scalar=scalar, in1=in1, op0=op0, op1=op1
                    )
                    OPLOG[ins.ins.name] = desc

                def scp(o, i, desc="scp"):
                    ins = nc.scalar.copy(out=o, in_=i)
                    OPLOG[ins.ins.name] = desc

                # u = v + pred (interleaved fp16, 2 element-halves so the
                # u-deint pieces can start early)
                half = 4 * fc
                vtt(ut[:, :half], vt[:, :half], pt[:, :half], ADD, "uaddV")
                vtt(ut[:, half:], vt[:, half:], pt[:, half:], ADD, "uaddV2")

                # deinterleave: comp (0,2,4,6 / 1,3,5,7) -> planar
                xv = sa.alloc(8)  # [x0,x1,x2,x3,y0,y1,y2,y3]
                uu = sa.alloc(8)  # [X0,X1,X2,X3,Y0,Y1,Y2,Y3]
                iv = vt[:, :].rearrange("p (f c g) -> p g c f", c=4, g=2)
                ov_ = R(xv, 8).rearrange("p (g c f) -> p g c f", c=4, g=2)
                scp(ov_[:, 0, :, :], iv[:, 0, :, :], desc="deint_vx")
                scp(ov_[:, 1, :, :], iv[:, 1, :, :], desc="deint_vy")
                # u-deint: first element-half on ScalarE, second on GPSIMD
                iu = ut[:, :].rearrange("p (f c g) -> p g c f", c=4, g=2)
                ou_ = R(uu, 8).rearrange("p (g c f) -> p g c f", c=4, g=2)
                hf = fc // 2
                scp(ou_[:, :, :, :hf], iu[:, :, :, :hf], desc="deint_u1")
                ins = nc.gpsimd.tensor_scalar_add(
                    out=ou_[:, :, :, hf:], in0=iu[:, :, :, hf:], scalar1=0.0
                )
                OPLOG[ins.ins.name] = "deint_u2g"

                # OT is element-interleaved (f*9 + c): out-DMA is contiguous
                ov = ot[:, :].rearrange("p (f c) -> p c f", c=9)
                ins = nc.gpsimd.memset(ov[:, 8, :], 1.0)
                OPLOG[ins.ins.name] = "ones_g"

                # diffs: D = [dx1,dx2,dx3,dy1,dy2,dy3]
                dd = sa.alloc(6)
                xv3 = V(xv, 8)
                vtt(V(dd, 6)[:, 0:3, :], xv3[:, 1:4, :], BC(xv, 3), SUB,
                    "diffx")
                vtt(V(dd, 6)[:, 3:6, :], xv3[:, 5:8, :], BC(xv + 4, 3), SUB,
                    "diffy")
                DX1, DX2, DX3, DY1, DY2, DY3 = range(dd, dd + 6)

                # n: n1=dx2dy3-dx3dy2, n2=dx3dy1-dx1dy3, n3=dx1dy2-dx2dy1
                pa = sa.alloc(3)
                pb = sa.alloc(3)
                for k, (a, b) in enumerate(((DX2, DY3), (DX3, DY1), (DX1, DY2))):
                    vtt(PL(pa + k), PL(a), PL(b), MUL, f"pa{k}")
                for k, (a, b) in enumerate(((DX3, DY2), (DX1, DY3), (DX2, DY1))):
                    vtt(PL(pb + k), PL(a), PL(b), MUL, f"pb{k}")
                ns = sa.alloc(4)  # fp16 [n0,n1,n2,n3]
                vtt(R(ns + 1, 3), R(pa, 3), R(pb, 3), SUB, "nsub")
                # fp32 n3 for the reciprocal (sub done at fp32 from fp16 in)
                vtt(R32(0, 1), PL(pa + 2), PL(pb + 2), SUB, "n3_32")
                t0 = sa.alloc(1)
                vtt(PL(t0), PL(ns + 1), PL(ns + 2), ADD, "t0")
                stt(PL(ns), PL(t0), -1.0, PL(ns + 3), MUL, SUB)  # n0=-(n1+n2)-n3
                # dots: per axis, planar groups [z0..z3, q0..q3, r0..r3]
                #   z_p = n_p W_p ; q_p = z_p x_p ; r_p = z_p y_p
                zx = sa.alloc(12)
                zy = sa.alloc(12)
                for zz, w in ((zx, 0), (zy, 4)):
                    vtt(V(zz, 12)[:, 0:4, :], V(ns, 4),
                        V(uu, 8)[:, w : w + 4, :], MUL, f"z{w}")
                    vtt(V(zz, 12)[:, 4:8, :], V(zz, 12)[:, 0:4, :],
                        V(xv, 8)[:, 0:4, :], MUL, f"q{w}")
                    vtt(V(zz, 12)[:, 8:12, :], V(zz, 12)[:, 0:4, :],
                        V(xv, 8)[:, 4:8, :], MUL, f"r{w}")
                # pairwise point sums: [3,4] -> [3,2] -> [3]
                tx = sa.alloc(6)
                ty = sa.alloc(6)
                for zz, td, nm in ((zx, tx, "TX"), (zy, ty, "TY")):
                    g = R(zz, 12).rearrange("p (g q f) -> p g q f", g=3, q=4)
                    vtt(R(td, 6).rearrange("p (g q f) -> p g q f", g=3, q=2),
                        g[:, :, 0:2, :], g[:, :, 2:4, :], ADD, nm)
                ss = sa.alloc(6)  # [aX,bX,cX,aY,bY,cY]
                for td, so, nm in ((tx, 0, "ssX"), (ty, 3, "ssY")):
                    tv = R(td, 6).rearrange("p (g q f) -> p g q f", g=3, q=2)
                    vtt(V(ss, 6)[:, so : so + 3, :], tv[:, :, 0, :],
                        tv[:, :, 1, :], ADD, nm)

                # 2x2: det = bXcY-bYcX, h6n = cXaY-cYaX, h7n = bYaX-bXaY
                AX, BX, CX, AY, BY, CY = range(ss, ss + 6)
                pc = sa.alloc(3)
                pd = sa.alloc(3)
                for k, (a, b) in enumerate(((BX, CY), (CX, AY), (BY, AX))):
                    vtt(PL(pc + k), PL(a), PL(b), MUL, f"pc{k}")
                for k, (a, b) in enumerate(((BY, CX), (CY, AX), (BX, AY))):
                    vtt(PL(pd + k), PL(a), PL(b), MUL, f"pd{k}")
                # det at fp32 (adjacent to n3_32), h6n/h7n at fp16
                vtt(R32(1, 1), PL(pc), PL(pd), SUB, "det32")
                dt67 = sa.alloc(2)
                vtt(V(dt67, 2), V(pc, 3)[:, 1:3, :], V(pd, 3)[:, 1:3, :], SUB,
                    "dt67")

                # fused reciprocal over [n3_32, det_32] -> [rD_32, rdet_32]
                nc.vector.reciprocal_approx_accurate(
                    out=R32(2, 2), in_=R32(0, 2), scratch=R32(4, 2)
                )
                # h6,h7 straight into the output staging (strided dst, 1x)
                vtt(ov[:, 6:8, :], V(dt67, 2), BC32(3, 2), MUL, "h67")

                # m = [x_p h6 (p=0..2), y_p h7 (p=0..2)]   (one fused op)
                m = sa.alloc(6)
                xv2 = R(xv, 8).rearrange("p (a q f) -> p a q f", a=2, q=4)
                h67b = (
                    ov[:, 6:8, :].unsqueeze(2).broadcast_to((PARTS, 2, 3, fc))
                )
                vtt(R(m, 6).rearrange("p (a q f) -> p a q f", a=2, q=3),
                    xv2[:, :, 0:3, :], h67b, MUL, "m12")
                # w = (m1 + 1) + m2  (fused +1, no ScalarE handoff)
                w = sa.alloc(3)
                stt(R(w, 3), R(m, 3), 1.0, R(m + 3, 3), ADD, ADD, "wfuse")
                # XW_p = w_p X_p, YW_p = w_p Y_p (p=0..2)   (one fused op)
                xw = sa.alloc(6)  # [XW0,XW1,XW2,YW0,YW1,YW2]
                wb = (
                    R(w, 3).rearrange("p (q f) -> p q f", f=fc)
                    .unsqueeze(1).broadcast_to((PARTS, 2, 3, fc))
                )
                uu2 = R(uu, 8).rearrange("p (a q f) -> p a q f", a=2, q=4)
                vtt(R(xw, 6).rearrange("p (a q f) -> p a q f", a=2, q=3),
                    wb, uu2[:, :, 0:3, :], MUL, "xwyw")

                # PQ = (XW1-XW0, XW2-XW0, YW1-YW0, YW2-YW0)
                pq = sa.alloc(4)
                xwv = R(xw, 6).rearrange("p (a b f) -> p a b f", a=2, b=3)
                vtt(
                    R(pq, 4).rearrange("p (a b f) -> p a b f", a=2, b=2),
                    xwv[:, :, 1:3, :],
                    xwv[:, :, 0, :].unsqueeze(2).broadcast_to((PARTS, 2, 2, fc)),
                    SUB,
                    "PQ",
                )

                # pE = (P1 dy2, Q1 dy2, dx1 P2, dx1 Q2)
                # pF = (P2 dy1, Q2 dy1, dx2 P1, dx2 Q1)
                pe = sa.alloc(4)
                pf = sa.alloc(4)
                pqv = V(pq, 4)
                vtt(V(pe, 4)[:, 0:2, :], pqv[:, 0:3:2, :], BC(DY2, 2), MUL,
                    "pe01")
                vtt(V(pe, 4)[:, 2:4, :], pqv[:, 1:4:2, :], BC(DX1, 2), MUL,
                    "pe23")
                vtt(V(pf, 4)[:, 0:2, :], pqv[:, 1:4:2, :], BC(DY1, 2), MUL,
                    "pf01")
                vtt(V(pf, 4)[:, 2:4, :], pqv[:, 0:3:2, :], BC(DX2, 2), MUL,
                    "pf23")
                hn = sa.alloc(4)  # [h0n, h3n, h1n, h4n]
                vtt(R(hn, 4), R(pe, 4), R(pf, 4), SUB, "hn")
                # h = hn * rD straight into output staging (c0,c3 then c1,c4)
                vtt(ov[:, 0:4:3, :], V(hn, 4)[:, 0:2, :], BC32(2, 2), MUL,
                    "hgA")
                vtt(ov[:, 1:5:3, :], V(hn, 4)[:, 2:4, :], BC32(2, 2), MUL,
                    "hgB")

                # h2 = XW0 - x0 h0 - y0 h1 ; h5 = YW0 - x0 h3 - y0 h4
                ee = sa.alloc(4)  # (x0 h0, y0 h1, x0 h3, y0 h4)
                xy0 = V(xv, 8)[:, 0:5:4, :]  # (x0, y0)
                vtt(V(ee, 4)[:, 0:2, :], xy0, ov[:, 0:2, :], MUL, "ee1")
                vtt(V(ee, 4)[:, 2:4, :], xy0, ov[:, 3:5, :], MUL, "ee2")
                s1 = sa.alloc(2)
                eev = V(ee, 4)
                vtt(V(s1, 2), V(xw, 6)[:, 0:4:3, :], eev[:, 0:3:2, :], SUB,
                    "s1")
                vtt(ov[:, 2:6:3, :], V(s1, 2), eev[:, 1:4:2, :], SUB, "h25")

                nc.sync.dma_start(
                    out=out[lo:hi, :].rearrange("(p f) c -> p (f c)", p=PARTS),
                    in_=ot[:, :],
                )
                lo = hi
    nc.finalize()
    return nc


_NC_CACHE = {}


def _get_nc():
    if "nc" not in _NC_CACHE:
        _NC_CACHE["nc"] = _build()
    return _NC_CACHE["nc"]


def kernel(pts_1_tile, pred_h4p_tile, _trace=False):
    pts = np.ascontiguousarray(
        np.asarray(pts_1_tile).reshape(B_TOTAL, 8).astype(np.float16)
    )
    prd = np.ascontiguousarray(
        np.asarray(pred_h4p_tile).reshape(B_TOTAL, 8).astype(np.float16)
    )
    nc = _get_nc()
    in_maps = [
        {
            "pts": pts[i * PER_CORE : (i + 1) * PER_CORE],
            "prd": prd[i * PER_CORE : (i + 1) * PER_CORE],
        }
        for i in range(N_CORES)
    ]
    res = run_bass_kernel_spmd(nc, in_maps, list(range(N_CORES)), trace=_trace)
    outs = np.concatenate([res.results[i]["out"] for i in range(N_CORES)], axis=0)
    H = outs.astype(np.float32).reshape(B_TOTAL, 3, 3)
    if _trace:
        return H, res
    return H


# revision 12
# speedup vs baseline: 1.5608x; 1.5608x over previous
"""Trainium2 Bass kernel: batched 4-point DLT homography (closed-form solve).

Contract: kernel(pts_1_tile, pred_h4p_tile) -> [B, 3, 3] float32, with
B = 524288 split across 8 NeuronCores (batch-parallel, no communication).

Math (per batch element, points p=0..3 with src (x_p,y_p), dst (X_p,Y_p)):
the DLT system rows are
    x h0 + y h1 + h2 = X (1 + x h6 + y h7)
    x h3 + y h4 + h5 = Y (1 + x h6 + y h7)
Eliminating (h0,h1,h2) from the four X-equations via the left null vector n
of M = [(x_p, y_p, 1)] gives one linear equation in (h6,h7); same for the
Y-equations. Solve the 2x2, back out the rest in closed form.

Layout: per-core 65536 elements as [128 partitions, 512 free], two
asymmetric chunks (128 + 384 columns) so chunk-0 compute starts once a
quarter of the input lands.  I/O is fp16 (host casts).  Planes are [128,fc]
scalars in a no-reuse slab (avoids WAR-hazard semaphores).  Engine split,
calibrated from HW traces: VectorE owns all math at fp16-2x (0.52 ns/elem);
ScalarE does the v-deinterleave + first u-half (fp16 copies run at only
~2 ns/elem on ACT); GPSIMD does the ones-memset + second u-half.  Final h
values are written by VectorE directly into the interleaved output staging
(strided dst, 1x) - no separate output copies.  Reciprocals of n3 and det
are fused into one two-plane fp32 op and consumed as fp32 broadcasts.
"""
import sys

for _p in ("/opt/trn_rl_repo", "/root/.axon_site/_ro/trn_rl_repo"):
    if _p not in sys.path:
        sys.path.append(_p)

import numpy as np

import concourse.bass as bass
import concourse.mybir as mybir
from concourse import bacc
from concourse.tile import TileContext
from concourse.bass_utils import run_bass_kernel_spmd

N_CORES = 8
B_TOTAL = 524288
PER_CORE = B_TOTAL // N_CORES  # 65536
PARTS = 128
F = PER_CORE // PARTS  # 512
CHUNKS = [128, 384]
FP32 = mybir.dt.float32
FP16 = mybir.dt.float16

ADD = mybir.AluOpType.add
SUB = mybir.AluOpType.subtract
MUL = mybir.AluOpType.mult


class _Slab:
    """Bump allocator, F-plane units, no reuse (avoids WAR semaphores)."""

    def __init__(self, nplanes):
        self.off = 0
        self.nplanes = nplanes

    def alloc(self, n):
        off = self.off
        self.off += n
        if self.off > self.nplanes:
            raise RuntimeError(f"slab OOM at {self.off}/{self.nplanes}")
        return off


OPLOG = {}


def _build():
    OPLOG.clear()
    nc = bacc.Bacc(None, target_bir_lowering=False, debug=True)
    pts = nc.dram_tensor("pts", [PER_CORE, 8], FP16, kind="ExternalInput")
    prd = nc.dram_tensor("prd", [PER_CORE, 8], FP16, kind="ExternalInput")
    out = nc.dram_tensor("out", [PER_CORE, 9], FP32, kind="ExternalOutput")

    N32 = 6   # fp32 slab: [n3_32, det_32, rD_32, rdet_32, scratch x2]
    NP = 128  # fp16 compute-plane slab (no reuse)

    with TileContext(nc) as tc:
        with tc.tile_pool(name="st", bufs=1) as pool:
            tiles = {}
            for c, fc in enumerate(CHUNKS):
                tiles[c] = {
                    "vt": pool.tile([PARTS, 8 * fc], FP16, tag=f"vt{c}", name=f"vt{c}"),
                    "pt": pool.tile([PARTS, 8 * fc], FP16, tag=f"pt{c}", name=f"pt{c}"),
                    "ut": pool.tile([PARTS, 8 * fc], FP16, tag=f"ut{c}", name=f"ut{c}"),
                    "ot": pool.tile([PARTS, 9 * fc], FP32, tag=f"ot{c}", name=f"ot{c}"),
                    "s32": pool.tile([PARTS, N32 * fc], FP32, tag=f"s32_{c}", name=f"s32_{c}"),
                    "sp": pool.tile([PARTS, NP * fc], FP16, tag=f"sp{c}", name=f"sp{c}"),
                }

            # All input DMAs up front, chunk 0 first (smallest => earliest start)
            lo = 0
            for c, fc in enumerate(CHUNKS):
                hi = lo + PARTS * fc
                nc.sync.dma_start(
                    out=tiles[c]["vt"][:, :],
                    in_=pts[lo:hi, :].rearrange("(p f) c -> p (f c)", p=PARTS),
                )
                nc.sync.dma_start(
                    out=tiles[c]["pt"][:, :],
                    in_=prd[lo:hi, :].rearrange("(p f) c -> p (f c)", p=PARTS),
                )
                lo = hi

            lo = 0
            for c, fc in enumerate(CHUNKS):
                hi = lo + PARTS * fc
                vt, pt, ut, ot = (tiles[c][k] for k in ("vt", "pt", "ut", "ot"))
                slab32, slabp = tiles[c]["s32"], tiles[c]["sp"]
                sa = _Slab(NP)

                def R32(off, n):
                    return slab32[:, off * fc : (off + n) * fc]

                def R(off, n):
                    return slabp[:, off * fc : (off + n) * fc]

                def V(off, n):
                    return R(off, n).rearrange("p (c f) -> p c f", f=fc)

                def PL(off):
                    return R(off, 1)

                def BC(off, k):
                    return PL(off).unsqueeze(1).broadcast_to((PARTS, k, fc))

                def BC32(off, k):
                    return (
                        R32(off, 1).unsqueeze(1).broadcast_to((PARTS, k, fc))
                    )

                def vtt(o, a, b, op, desc=""):
                    ins = nc.vector.tensor_tensor(out=o, in0=a, in1=b, op=op)
                    OPLOG[ins.ins.name] = desc or "tt"

                def gtt(o, a, b, op, desc=""):
                    ins = nc.gpsimd.tensor_tensor(out=o, in0=a, in1=b, op=op)
                    OPLOG[ins.ins.name] = desc or "gtt"

                def stt(o, in0, scalar, in1, op0, op1, desc="stt"):
                    ins = nc.vector.scalar_tensor_tensor(
                        out=o, in0=in0, scalar=scalar, in1=in1, op0=op0, op1=op1
                    )
                    OPLOG[ins.ins.name] = desc

                def scp(o, i, desc="scp"):
                    ins = nc.scalar.copy(out=o, in_=i)
                    OPLOG[ins.ins.name] = desc

                # u = v + pred (interleaved fp16, 2 element-halves so the
                # u-deint pieces can start early)
                half = 4 * fc
                vtt(ut[:, :half], vt[:, :half], pt[:, :half], ADD, "uaddV")
                vtt(ut[:, half:], vt[:, half:], pt[:, half:], ADD, "uaddV2")

                # deinterleave: comp (0,2,4,6 / 1,3,5,7) -> planar
                xv = sa.alloc(8)  # [x0,x1,x2,x3,y0,y1,y2,y3]
                uu = sa.alloc(8)  # [X0,X1,X2,X3,Y0,Y1,Y2,Y3]
                iv = vt[:, :].rearrange("p (f c g) -> p g c f", c=4, g=2)
                ov_ = R(xv, 8).rearrange("p (g c f) -> p g c f", c=4, g=2)
                scp(ov_[:, 0, :, :], iv[:, 0, :, :], desc="deint_vx")
                scp(ov_[:, 1, :, :], iv[:, 1, :, :], desc="deint_vy")
                # u-deint: first element-half on ScalarE, second on GPSIMD
                iu = ut[:, :].rearrange("p (f c g) -> p g c f", c=4, g=2)
                ou_ = R(uu, 8).rearrange("p (g c f) -> p g c f", c=4, g=2)
                hf = fc // 2
                scp(ou_[:, :, :, :hf], iu[:, :, :, :hf], desc="deint_u1")
                scp(ou_[:, :, :, hf:], iu[:, :, :, hf:], desc="deint_u2")

                # OT is element-interleaved (f*9 + c): out-DMA is contiguous
                ov = ot[:, :].rearrange("p (f c) -> p c f", c=9)
                ins = nc.gpsimd.memset(ov[:, 8, :], 1.0)
                OPLOG[ins.ins.name] = "ones_g"

                # diffs: D = [dx1,dx2,dx3,dy1,dy2,dy3]
                dd = sa.alloc(6)
                xv3 = V(xv, 8)
                vtt(V(dd, 6)[:, 0:3, :], xv3[:, 1:4, :], BC(xv, 3), SUB,
                    "diffx")
                vtt(V(dd, 6)[:, 3:6, :], xv3[:, 5:8, :], BC(xv + 4, 3), SUB,
                    "diffy")
                DX1, DX2, DX3, DY1, DY2, DY3 = range(dd, dd + 6)

                # n: n1=dx2dy3-dx3dy2, n2=dx3dy1-dx1dy3, n3=dx1dy2-dx2dy1
                pa = sa.alloc(3)
                pb = sa.alloc(3)
                for k, (a, b) in enumerate(((DX2, DY3), (DX3, DY1), (DX1, DY2))):
                    vtt(PL(pa + k), PL(a), PL(b), MUL, f"pa{k}")
                for k, (a, b) in enumerate(((DX3, DY2), (DX1, DY3), (DX2, DY1))):
                    vtt(PL(pb + k), PL(a), PL(b), MUL, f"pb{k}")
                ns = sa.alloc(4)  # fp16 [n0,n1,n2,n3]
                vtt(R(ns + 1, 3), R(pa, 3), R(pb, 3), SUB, "nsub")
                # fp32 n3 for the reciprocal (sub done at fp32 from fp16 in)
                vtt(R32(0, 1), PL(pa + 2), PL(pb + 2), SUB, "n3_32")
                t0 = sa.alloc(1)
                vtt(PL(t0), PL(ns + 1), PL(ns + 2), ADD, "t0")
                stt(PL(ns), PL(t0), -1.0, PL(ns + 3), MUL, SUB)  # n0=-(n1+n2)-n3
                # dots: per axis, planar groups [z0..z3, q0..q3, r0..r3]
                #   z_p = n_p W_p ; q_p = z_p x_p ; r_p = z_p y_p
                zx = sa.alloc(12)
                zy = sa.alloc(12)
                for zz, w in ((zx, 0), (zy, 4)):
                    rtt = gtt if w == 0 else vtt
                    vtt(V(zz, 12)[:, 0:4, :], V(ns, 4),
                        V(uu, 8)[:, w : w + 4, :], MUL, f"z{w}")
                    vtt(V(zz, 12)[:, 4:8, :], V(zz, 12)[:, 0:4, :],
                        V(xv, 8)[:, 0:4, :], MUL, f"q{w}")
                    rtt(V(zz, 12)[:, 8:12, :], V(zz, 12)[:, 0:4, :],
                        V(xv, 8)[:, 4:8, :], MUL, f"r{w}")
                # pairwise point sums: [3,4] -> [3,2] -> [3]
                tx = sa.alloc(6)
                ty = sa.alloc(6)
                for zz, td, nm in ((zx, tx, "TX"), (zy, ty, "TY")):
                    g = R(zz, 12).rearrange("p (g q f) -> p g q f", g=3, q=4)
                    vtt(R(td, 6).rearrange("p (g q f) -> p g q f", g=3, q=2),
                        g[:, :, 0:2, :], g[:, :, 2:4, :], ADD, nm)
                ss = sa.alloc(6)  # [aX,bX,cX,aY,bY,cY]
                for td, so, nm in ((tx, 0, "ssX"), (ty, 3, "ssY")):
                    tv = R(td, 6).rearrange("p (g q f) -> p g q f", g=3, q=2)
                    vtt(V(ss, 6)[:, so : so + 3, :], tv[:, :, 0, :],
                        tv[:, :, 1, :], ADD, nm)

                # 2x2: det = bXcY-bYcX, h6n = cXaY-cYaX, h7n = bYaX-bXaY
                AX, BX, CX, AY, BY, CY = range(ss, ss + 6)
                pc = sa.alloc(3)
                pd = sa.alloc(3)
                for k, (a, b) in enumerate(((BX, CY), (CX, AY), (BY, AX))):
                    vtt(PL(pc + k), PL(a), PL(b), MUL, f"pc{k}")
                for k, (a, b) in enumerate(((BY, CX), (CY, AX), (BX, AY))):
                    vtt(PL(pd + k), PL(a), PL(b), MUL, f"pd{k}")
                # det at fp32 (adjacent to n3_32), h6n/h7n at fp16
                vtt(R32(1, 1), PL(pc), PL(pd), SUB, "det32")
                dt67 = sa.alloc(2)
                vtt(V(dt67, 2), V(pc, 3)[:, 1:3, :], V(pd, 3)[:, 1:3, :], SUB,
                    "dt67")

                # fused reciprocal over [n3_32, det_32] -> [rD_32, rdet_32]
                nc.vector.reciprocal_approx_accurate(
                    out=R32(2, 2), in_=R32(0, 2), scratch=R32(4, 2)
                )
                h67 = sa.alloc(2)
                vtt(V(h67, 2), V(dt67, 2), BC32(3, 2), MUL, "h67")
                scp(ov[:, 6:8, :], V(h67, 2), desc="h67cp")

                # m = [x_p h6 (p=0..2), y_p h7 (p=0..2)]   (one fused op)
                m = sa.alloc(6)
                xv2 = R(xv, 8).rearrange("p (a q f) -> p a q f", a=2, q=4)
                h67b = (
                    V(h67, 2).unsqueeze(2).broadcast_to((PARTS, 2, 3, fc))
                )
                vtt(R(m, 6).rearrange("p (a q f) -> p a q f", a=2, q=3),
                    xv2[:, :, 0:3, :], h67b, MUL, "m12")
                # w = (m1 + 1) + m2  (fused +1, no ScalarE handoff)
                w = sa.alloc(3)
                stt(R(w, 3), R(m, 3), 1.0, R(m + 3, 3), ADD, ADD, "wfuse")
                # XW_p = w_p X_p, YW_p = w_p Y_p (p=0..2)   (one fused op)
                xw = sa.alloc(6)  # [XW0,XW1,XW2,YW0,YW1,YW2]
                wb = (
                    R(w, 3).rearrange("p (q f) -> p q f", f=fc)
                    .unsqueeze(1).broadcast_to((PARTS, 2, 3, fc))
                )
                uu2 = R(uu, 8).rearrange("p (a q f) -> p a q f", a=2, q=4)
                vtt(R(xw, 6).rearrange("p (a q f) -> p a q f", a=2, q=3),
                    wb, uu2[:, :, 0:3, :], MUL, "xwyw")

                # PQ = (XW1-XW0, XW2-XW0, YW1-YW0, YW2-YW0)
                pq = sa.alloc(4)
                xwv = R(xw, 6).rearrange("p (a b f) -> p a b f", a=2, b=3)
                vtt(
                    R(pq, 4).rearrange("p (a b f) -> p a b f", a=2, b=2),
                    xwv[:, :, 1:3, :],
                    xwv[:, :, 0, :].unsqueeze(2).broadcast_to((PARTS, 2, 2, fc)),
                    SUB,
                    "PQ",
                )

                # pE = (P1 dy2, Q1 dy2, dx1 P2, dx1 Q2)
                # pF = (P2 dy1, Q2 dy1, dx2 P1, dx2 Q1)
                pe = sa.alloc(4)
                pf = sa.alloc(4)
                pqv = V(pq, 4)
                vtt(V(pe, 4)[:, 0:2, :], pqv[:, 0:3:2, :], BC(DY2, 2), MUL,
                    "pe01")
                vtt(V(pe, 4)[:, 2:4, :], pqv[:, 1:4:2, :], BC(DX1, 2), MUL,
                    "pe23")
                vtt(V(pf, 4)[:, 0:2, :], pqv[:, 1:4:2, :], BC(DY1, 2), MUL,
                    "pf01")
                vtt(V(pf, 4)[:, 2:4, :], pqv[:, 0:3:2, :], BC(DX2, 2), MUL,
                    "pf23")
                hn = sa.alloc(4)  # [h0n, h3n, h1n, h4n]
                vtt(R(hn, 4), R(pe, 4), R(pf, 4), SUB, "hn")
                hg = sa.alloc(4)  # [h0, h3, h1, h4]
                vtt(V(hg, 4), V(hn, 4), BC32(2, 4), MUL, "hg")
                scp(ov[:, 0:4:3, :], V(hg, 2), desc="hgAcp")
                scp(ov[:, 1:5:3, :], V(hg + 2, 2), desc="hgBcp")

                # h2 = XW0 - x0 h0 - y0 h1 ; h5 = YW0 - x0 h3 - y0 h4
                ee = sa.alloc(4)  # (x0 h0, y0 h1, x0 h3, y0 h4)
                xy0 = V(xv, 8)[:, 0:5:4, :]  # (x0, y0)
                hgv = V(hg, 4)
                vtt(V(ee, 4)[:, 0:2, :], xy0, hgv[:, 0:3:2, :], MUL, "ee1")
                vtt(V(ee, 4)[:, 2:4, :], xy0, hgv[:, 1:4:2, :], MUL, "ee2")
                s1 = sa.alloc(2)
                eev = V(ee, 4)
                vtt(V(s1, 2), V(xw, 6)[:, 0:4:3, :], eev[:, 0:3:2, :], SUB,
                    "s1")
                h25 = sa.alloc(2)
                vtt(V(h25, 2), V(s1, 2), eev[:, 1:4:2, :], SUB, "h25")
                # tail: copy + DMA in f-halves so the first half's store
                # overlaps the second half's copy
                oh = out[lo:hi, :].rearrange("(p f) c -> p (f c)", p=PARTS)
                for k in range(2):
                    fl, fh = k * hf, (k + 1) * hf
                    scp(ov[:, 2:6:3, fl:fh], V(h25, 2)[:, :, fl:fh],
                        desc=f"h25cp{k}")
                    nc.sync.dma_start(
                        out=oh[:, 9 * fl : 9 * fh],
                        in_=ot[:, 9 * fl : 9 * fh],
                    )
                lo = hi
    nc.finalize()
    return nc


_NC_CACHE = {}


def _get_nc():
    if "nc" not in _NC_CACHE:
        _NC_CACHE["nc"] = _build()
    return _NC_CACHE["nc"]


def kernel(pts_1_tile, pred_h4p_tile, _trace=False):
    pts = np.ascontiguousarray(
        np.asarray(pts_1_tile).reshape(B_TOTAL, 8).astype(np.float16)
    )
    prd = np.ascontiguousarray(
        np.asarray(pred_h4p_tile).reshape(B_TOTAL, 8).astype(np.float16)
    )
    nc = _get_nc()
    in_maps = [
        {
            "pts": pts[i * PER_CORE : (i + 1) * PER_CORE],
            "prd": prd[i * PER_CORE : (i + 1) * PER_CORE],
        }
        for i in range(N_CORES)
    ]
    res = run_bass_kernel_spmd(nc, in_maps, list(range(N_CORES)), trace=_trace)
    outs = np.concatenate([res.results[i]["out"] for i in range(N_CORES)], axis=0)
    H = np.ascontiguousarray(outs).reshape(B_TOTAL, 3, 3)
    if _trace:
        return H, res
    return H


# revision 14
# speedup vs baseline: 1.9089x; 1.2230x over previous
"""Trainium2 Bass kernel: batched 4-point DLT homography (closed-form solve).

Contract: kernel(pts_1_tile, pred_h4p_tile) -> [B, 3, 3] float32, with
B = 524288 split across 8 NeuronCores (batch-parallel, no communication).

Math (per batch element, points p=0..3 with src (x_p,y_p), dst (X_p,Y_p)):
the DLT system rows are
    x h0 + y h1 + h2 = X (1 + x h6 + y h7)
    x h3 + y h4 + h5 = Y (1 + x h6 + y h7)
Eliminating (h0,h1,h2) from the four X-equations via the left null vector n
of M = [(x_p, y_p, 1)] gives one linear equation in (h6,h7); same for the
Y-equations. Solve the 2x2, back out the rest in closed form.

Layout: per-core 65536 elements as [128 partitions, 512 free], two
asymmetric chunks so chunk-0 compute starts early.  The HOST pre-transposes
inputs to planar component planes packed per-chunk (contiguous 2D DMA at
full rate) and post-transposes the planar fp16 output back to [B,3,3]
fp32 (+ the constant ones column) — so the device does zero shuffling:
no deinterleave, no output staging copies.  ScalarE/GPSIMD stay idle
(their SBUF traffic would slow concurrent DVE ops); VectorE runs all math
as fp16-2x multi-plane ops over [128,fc] planes in a no-reuse slab.
Dot products use sum(n * {1,x,y}) = 0:  a/b/c = sum_{p>=1} n_p {1,x,y}_p
(U_p - U_0), 9 products per axis, no n0.  One reciprocal_approx_fast
(~18 bits, plenty next to fp16) covers 1/n3 and 1/det in one op.
"""
import sys

for _p in ("/opt/trn_rl_repo", "/root/.axon_site/_ro/trn_rl_repo"):
    if _p not in sys.path:
        sys.path.append(_p)

import numpy as np

import concourse.bass as bass
import concourse.mybir as mybir
from concourse import bacc
from concourse.tile import TileContext
from concourse.bass_utils import run_bass_kernel_spmd

N_CORES = 8
B_TOTAL = 524288
PER_CORE = B_TOTAL // N_CORES  # 65536
PARTS = 128
F = PER_CORE // PARTS  # 512
CHUNKS = [96, 416]
FP32 = mybir.dt.float32
FP16 = mybir.dt.float16

ADD = mybir.AluOpType.add
SUB = mybir.AluOpType.subtract
MUL = mybir.AluOpType.mult

# component order of the planar input planes: x0..x3, y0..y3
CORDER = [0, 2, 4, 6, 1, 3, 5, 7]


class _Slab:
    """Bump allocator, F-plane units, no reuse (avoids WAR semaphores)."""

    def __init__(self, nplanes):
        self.off = 0
        self.nplanes = nplanes

    def alloc(self, n):
        off = self.off
        self.off += n
        if self.off > self.nplanes:
            raise RuntimeError(f"slab OOM at {self.off}/{self.nplanes}")
        return off


OPLOG = {}


def _build():
    OPLOG.clear()
    nc = bacc.Bacc(None, target_bir_lowering=False, debug=True)
    # planar, chunk-blocked: per partition [8 planes x fc] per chunk
    pts = nc.dram_tensor("pts", [PARTS, 8 * F], FP16, kind="ExternalInput")
    prd = nc.dram_tensor("prd", [PARTS, 8 * F], FP16, kind="ExternalInput")
    out = nc.dram_tensor("out", [PARTS, 8 * F], FP16, kind="ExternalOutput")

    N32 = 4   # fp32 slab: [n3_32, det_32, rD_32, rdet_32]
    NP = 130  # fp16 compute-plane slab (no reuse)

    with TileContext(nc) as tc:
        with tc.tile_pool(name="st", bufs=1) as pool:
            slabs = {}
            for c, fc in enumerate(CHUNKS):
                slabs[c] = (
                    pool.tile([PARTS, NP * fc], FP16, tag=f"sp{c}", name=f"sp{c}"),
                    pool.tile([PARTS, N32 * fc], FP32, tag=f"s32_{c}", name=f"s32_{c}"),
                )

            # xv/pp are the first 16 slab planes; DMA inputs straight in.
            # All input DMAs up front, chunk 0 first (smallest).
            off = 0
            for c, fc in enumerate(CHUNKS):
                nc.sync.dma_start(
                    out=slabs[c][0][:, : 8 * fc],
                    in_=pts[:, off : off + 8 * fc],
                )
                nc.sync.dma_start(
                    out=slabs[c][0][:, 8 * fc : 16 * fc],
                    in_=prd[:, off : off + 8 * fc],
                )
                off += 8 * fc

            off = 0
            for c, fc in enumerate(CHUNKS):
                slabp, slab32 = slabs[c]
                sa = _Slab(NP)

                def R32(o, n):
                    return slab32[:, o * fc : (o + n) * fc]

                def R(o, n):
                    return slabp[:, o * fc : (o + n) * fc]

                def V(o, n):
                    return R(o, n).rearrange("p (c f) -> p c f", f=fc)

                def PL(o):
                    return R(o, 1)

                def BC(o, k):
                    return PL(o).unsqueeze(1).broadcast_to((PARTS, k, fc))

                def vtt(o, a, b, op, desc=""):
                    ins = nc.vector.tensor_tensor(out=o, in0=a, in1=b, op=op)
                    OPLOG[ins.ins.name] = desc or "tt"

                def stt(o, in0, scalar, in1, op0, op1, desc="stt"):
                    ins = nc.vector.scalar_tensor_tensor(
                        out=o, in0=in0, scalar=scalar, in1=in1, op0=op0, op1=op1
                    )
                    OPLOG[ins.ins.name] = desc

                xv = sa.alloc(8)  # [x0,x1,x2,x3,y0,y1,y2,y3]  (DMA'd)
                pp = sa.alloc(8)  # pred planar, same order     (DMA'd)
                uu = sa.alloc(8)  # [X0..X3, Y0..Y3] = xv + pp

                # diffs dd = [dx1,dx2,dx3,dy1,dy2,dy3] in one fused op
                dd = sa.alloc(6)
                xv2 = R(xv, 8).rearrange("p (a q f) -> p a q f", a=2, q=4)
                vtt(R(dd, 6).rearrange("p (a q f) -> p a q f", a=2, q=3),
                    xv2[:, :, 1:4, :],
                    xv2[:, :, 0, :].unsqueeze(2).broadcast_to((PARTS, 2, 3, fc)),
                    SUB, "diffs")
                DX1, DX2, DX3, DY1, DY2, DY3 = range(dd, dd + 6)

                # n: n1=dx2dy3-dx3dy2, n2=dx3dy1-dx1dy3, n3=dx1dy2-dx2dy1
                pa = sa.alloc(3)
                pb = sa.alloc(3)
                for k, (a, b) in enumerate(((DX2, DY3), (DX3, DY1), (DX1, DY2))):
                    vtt(PL(pa + k), PL(a), PL(b), MUL, f"pa{k}")
                for k, (a, b) in enumerate(((DX3, DY2), (DX1, DY3), (DX2, DY1))):
                    vtt(PL(pb + k), PL(a), PL(b), MUL, f"pb{k}")
                ns = sa.alloc(3)  # fp16 [n1,n2,n3]
                vtt(R(ns, 3), R(pa, 3), R(pb, 3), SUB, "nsub")
                # fp32 n3 for the reciprocal (sub done at fp32 from fp16 in)
                vtt(R32(0, 1), PL(pa + 2), PL(pb + 2), SUB, "n3_32")

                # uu = xv + pred (planar, one op); UD_p = U_p - U_0 (p=1..3)
                vtt(R(uu, 8), R(xv, 8), R(pp, 8), ADD, "uadd")
                uu2 = R(uu, 8).rearrange("p (a q f) -> p a q f", a=2, q=4)
                ud = sa.alloc(6)  # [UX1,UX2,UX3,UY1,UY2,UY3]
                vtt(R(ud, 6).rearrange("p (a q f) -> p a q f", a=2, q=3),
                    uu2[:, :, 1:4, :],
                    uu2[:, :, 0, :].unsqueeze(2).broadcast_to((PARTS, 2, 3, fc)),
                    SUB, "udiff")

                # dots via sum(n)=0:  a = sum n_p UD_p, b = sum n_p x_p UD_p,
                # c = sum n_p y_p UD_p  (p=1..3).  Planar [z1..3, q1..3, r1..3]
                zx = sa.alloc(9)
                zy = sa.alloc(9)
                for zz, w in ((zx, 0), (zy, 3)):
                    vtt(V(zz, 9)[:, 0:3, :], V(ns, 3),
                        V(ud, 6)[:, w : w + 3, :], MUL, f"z{w}")
                    vtt(V(zz, 9)[:, 3:6, :], V(zz, 9)[:, 0:3, :],
                        V(xv, 8)[:, 1:4, :], MUL, f"q{w}")
                    vtt(V(zz, 9)[:, 6:9, :], V(zz, 9)[:, 0:3, :],
                        V(xv, 8)[:, 5:8, :], MUL, f"r{w}")
                # sums over p: (g0+g1)+g2 per (a,b,c) group
                s2x = sa.alloc(3)
                s2y = sa.alloc(3)
                for zz, sd in ((zx, s2x), (zy, s2y)):
                    g = R(zz, 9).rearrange("p (g q f) -> p g q f", g=3, q=3)
                    vtt(R(sd, 3).rearrange("p (g f) -> p g f", g=3),
                        g[:, :, 0, :], g[:, :, 1, :], ADD, "s2")
                ss = sa.alloc(6)  # [aX,bX,cX,aY,bY,cY]
                for zz, sd, so in ((zx, s2x, 0), (zy, s2y, 3)):
                    g = R(zz, 9).rearrange("p (g q f) -> p g q f", g=3, q=3)
                    vtt(V(ss, 6)[:, so : so + 3, :],
                        R(sd, 3).rearrange("p (g f) -> p g f", g=3),
                        g[:, :, 2, :], ADD, "ss")

                # 2x2: det = bXcY-bYcX, h6n = cXaY-cYaX, h7n = bYaX-bXaY
                AX, BX, CX, AY, BY, CY = range(ss, ss + 6)
                pc = sa.alloc(3)
                pd = sa.alloc(3)
                for k, (a, b) in enumerate(((BX, CY), (CX, AY), (BY, AX))):
                    vtt(PL(pc + k), PL(a), PL(b), MUL, f"pc{k}")
                for k, (a, b) in enumerate(((BY, CX), (CY, AX), (BX, AY))):
                    vtt(PL(pd + k), PL(a), PL(b), MUL, f"pd{k}")
                # det at fp32 (adjacent to n3_32), h6n/h7n at fp16
                vtt(R32(1, 1), PL(pc), PL(pd), SUB, "det32")
                dt67 = sa.alloc(2)
                vtt(V(dt67, 2), V(pc, 3)[:, 1:3, :], V(pd, 3)[:, 1:3, :], SUB,
                    "dt67")

                # fused reciprocal over [n3_32, det_32] -> [rD_32, rdet_32]
                ins = nc.vector.reciprocal_approx_fast(
                    out=R32(2, 2), in_=R32(0, 2)
                )
                OPLOG[ins.ins.name] = "recip"
                rc = sa.alloc(2)  # fp16 [rD, rdet]
                ins = nc.vector.tensor_scalar_add(
                    out=R(rc, 2), in0=R32(2, 2), scalar1=0.0
                )
                OPLOG[ins.ins.name] = "rcast"

                # output staging: planar fp16 planes [h0..h7]
                oo = sa.alloc(8)
                ov = V(oo, 8)
                vtt(ov[:, 6:8, :], V(dt67, 2), BC(rc + 1, 2), MUL, "h67")

                # m = [x_p h6 (p=0..2), y_p h7 (p=0..2)]   (one fused op)
                m = sa.alloc(6)
                h67b = (
                    ov[:, 6:8, :].unsqueeze(2).broadcast_to((PARTS, 2, 3, fc))
                )
                vtt(R(m, 6).rearrange("p (a q f) -> p a q f", a=2, q=3),
                    xv2[:, :, 0:3, :], h67b, MUL, "m12")
                # w = (m1 + 1) + m2  (fused +1)
                w = sa.alloc(3)
                stt(R(w, 3), R(m, 3), 1.0, R(m + 3, 3), ADD, ADD, "wfuse")
                # XW_p = w_p X_p, YW_p = w_p Y_p (p=0..2)   (one fused op)
                xw = sa.alloc(6)  # [XW0,XW1,XW2,YW0,YW1,YW2]
                wb = (
                    R(w, 3).rearrange("p (q f) -> p q f", f=fc)
                    .unsqueeze(1).broadcast_to((PARTS, 2, 3, fc))
                )
                vtt(R(xw, 6).rearrange("p (a q f) -> p a q f", a=2, q=3),
                    wb, uu2[:, :, 0:3, :], MUL, "xwyw")

                # PQ = (XW1-XW0, XW2-XW0, YW1-YW0, YW2-YW0)
                pq = sa.alloc(4)
                xwv = R(xw, 6).rearrange("p (a b f) -> p a b f", a=2, b=3)
                vtt(
                    R(pq, 4).rearrange("p (a b f) -> p a b f", a=2, b=2),
                    xwv[:, :, 1:3, :],
                    xwv[:, :, 0, :].unsqueeze(2).broadcast_to((PARTS, 2, 2, fc)),
                    SUB,
                    "PQ",
                )

                # pE = (P1 dy2, Q1 dy2, dx1 P2, dx1 Q2)
                # pF = (P2 dy1, Q2 dy1, dx2 P1, dx2 Q1)
                pe = sa.alloc(4)
                pf = sa.alloc(4)
                pqv = V(pq, 4)
                vtt(V(pe, 4)[:, 0:2, :], pqv[:, 0:3:2, :], BC(DY2, 2), MUL,
                    "pe01")
                vtt(V(pe, 4)[:, 2:4, :], pqv[:, 1:4:2, :], BC(DX1, 2), MUL,
                    "pe23")
                vtt(V(pf, 4)[:, 0:2, :], pqv[:, 1:4:2, :], BC(DY1, 2), MUL,
                    "pf01")
                vtt(V(pf, 4)[:, 2:4, :], pqv[:, 0:3:2, :], BC(DX2, 2), MUL,
                    "pf23")
                hn = sa.alloc(4)  # [h0n, h3n, h1n, h4n]
                vtt(R(hn, 4), R(pe, 4), R(pf, 4), SUB, "hn")
                # h = hn * rD into the output planes (plane-strided dst is
                # contiguous per-plane, full rate)
                vtt(ov[:, 0:4:3, :], V(hn, 4)[:, 0:2, :], BC(rc, 2), MUL,
                    "hgA")
                vtt(ov[:, 1:5:3, :], V(hn, 4)[:, 2:4, :], BC(rc, 2), MUL,
                    "hgB")

                # h2 = XW0 - x0 h0 - y0 h1 ; h5 = YW0 - x0 h3 - y0 h4
                ee = sa.alloc(4)  # (x0 h0, y0 h1, x0 h3, y0 h4)
                xy0 = V(xv, 8)[:, 0:5:4, :]  # (x0, y0)
                vtt(V(ee, 4)[:, 0:2, :], xy0, ov[:, 0:2, :], MUL, "ee1")
                vtt(V(ee, 4)[:, 2:4, :], xy0, ov[:, 3:5, :], MUL, "ee2")
                s1 = sa.alloc(2)
                eev = V(ee, 4)
                vtt(V(s1, 2), V(xw, 6)[:, 0:4:3, :], eev[:, 0:3:2, :], SUB,
                    "s1")
                vtt(ov[:, 2:6:3, :], V(s1, 2), eev[:, 1:4:2, :], SUB, "h25")

                nc.sync.dma_start(
                    out=out[:, off : off + 8 * fc],
                    in_=R(oo, 8),
                )
                off += 8 * fc
    nc.finalize()
    return nc


_NC_CACHE = {}


def _get_nc():
    if "nc" not in _NC_CACHE:
        _NC_CACHE["nc"] = _build()
    return _NC_CACHE["nc"]


def _pack(arr):
    """[PER_CORE, 8] fp16 -> [PARTS, 8*F] planar chunk-blocked."""
    a = arr.reshape(PARTS, F, 8)[:, :, CORDER].transpose(0, 2, 1)  # [p,c,f]
    lo = 0
    blocks = []
    for fc in CHUNKS:
        blocks.append(a[:, :, lo : lo + fc].reshape(PARTS, 8 * fc))
        lo += fc
    return np.ascontiguousarray(np.concatenate(blocks, axis=1))


def _unpack(o):
    """[PARTS, 8*F] planar chunk-blocked fp16 -> [PER_CORE, 8] fp32."""
    cols = np.empty((PARTS, 8, F), np.float16)
    lo = co = 0
    for fc in CHUNKS:
        cols[:, :, lo : lo + fc] = o[:, co : co + 8 * fc].reshape(PARTS, 8, fc)
        lo += fc
        co += 8 * fc
    return cols.transpose(0, 2, 1).reshape(PER_CORE, 8).astype(np.float32)


def kernel(pts_1_tile, pred_h4p_tile, _trace=False):
    pts = np.asarray(pts_1_tile).reshape(B_TOTAL, 8).astype(np.float16)
    prd = np.asarray(pred_h4p_tile).reshape(B_TOTAL, 8).astype(np.float16)
    nc = _get_nc()
    in_maps = [
        {
            "pts": _pack(pts[i * PER_CORE : (i + 1) * PER_CORE]),
            "prd": _pack(prd[i * PER_CORE : (i + 1) * PER_CORE]),
        }
        for i in range(N_CORES)
    ]
    res = run_bass_kernel_spmd(nc, in_maps, list(range(N_CORES)), trace=_trace)
    H = np.ones((B_TOTAL, 9), np.float32)
    for i in range(N_CORES):
        H[i * PER_CORE : (i + 1) * PER_CORE, :8] = _unpack(res.results[i]["out"])
    H = H.reshape(B_TOTAL, 3, 3)
    if _trace:
        return H, res
    return H
